# revision 1
# baseline (speedup 1.0000x reference)
"""Quantized-weight batched linear: out[b,n,m] = sum_k deq(qweight)[n,k] * x[b,k,m].

Strategy:
  - Host: dequantize weight (fp32, exact same formula as the oracle), transpose
    to (K, N), round weights + activations to bf16.
  - Device (8 cores, data-parallel over batch B=64 -> 8 batches/core):
    PE bf16 matmuls, K accumulated in PSUM over 8 chunks of 128,
    N tiled 8x128 (PSUM partitions), M tiled 2x512 (PSUM bank free-dim).
  - Gather core outputs along batch -> (64, 1024, 1024) fp32.
"""

import numpy as np
import ml_dtypes

N = 1024  # output rows (weight rows)
K = 1024  # reduction dim
M = 1024  # columns of x per batch
NGROUP = 16
GS = K // NGROUP
B = 64
NCORES = 8
BPC = B // NCORES  # batches per core

_CACHE = {}
LAST_RESULT = None  # BassKernelResults of the most recent run (for profiling)


def _build_nc(bpc=BPC, k=K, n=N, m=M):
    import concourse.mybir as mybir
    import concourse.tile as tile
    from concourse import bacc

    kc = k // 128   # contraction chunks (partition dim)
    nt = n // 128   # output-row tiles (PSUM partition dim)
    mt = m // 512   # moving free-dim tiles (one PSUM bank each)

    nc = bacc.Bacc(None, target_bir_lowering=False, debug=False)
    wt = nc.dram_tensor("wt", [k, n], mybir.dt.bfloat16, kind="ExternalInput")
    xs = nc.dram_tensor("xs", [bpc, k, m], mybir.dt.bfloat16, kind="ExternalInput")
    out = nc.dram_tensor("out", [bpc, n, m], mybir.dt.float32, kind="ExternalOutput")

    nh = 2           # process each batch in nh groups of n0 tiles
    npg = nt // nh   # n0 tiles per group; npg*mt PSUM banks live at once

    with tile.TileContext(nc) as tc:
        with (
            tc.tile_pool(name="wpool", bufs=1) as wpool,
            tc.tile_pool(name="xpool", bufs=2 * kc) as xpool,
            tc.tile_pool(name="opool", bufs=8) as opool,
            tc.tile_pool(name="psum", bufs=8, space="PSUM") as psum_pool,
        ):
            # All loads on the sync HWDGE queue, all stores on the scalar
            # HWDGE queue (static DMAs occupy the issuing sequencer for the
            # transfer; separate streams avoid head-of-line blocking and
            # Tile's cross-queue ordering waits).
            #
            # Startup: interleave weight slab k with x[batch0] chunk k so
            # chunk k's matmuls (k-outer order below gives 1.7us of PE work
            # per chunk) never wait on later transfers.
            wsb = []
            xcur = []
            for kk in range(kc):
                wtile = wpool.tile([128, n], mybir.dt.bfloat16, tag=f"w{kk}", name=f"w{kk}")
                nc.sync.dma_start(out=wtile[:], in_=wt[kk * 128:(kk + 1) * 128, :])
                wsb.append(wtile)
                xt = xpool.tile([128, m], mybir.dt.bfloat16, tag="x", name=f"x0_{kk}")
                nc.sync.dma_start(out=xt[:], in_=xs[0, kk * 128:(kk + 1) * 128, :])
                xcur.append(xt)

            for b in range(bpc):
                if b + 1 < bpc:
                    xnext = []
                    for kk in range(kc):
                        xt = xpool.tile([128, m], mybir.dt.bfloat16, tag="x", name=f"x{b + 1}_{kk}")
                        nc.sync.dma_start(out=xt[:], in_=xs[b + 1, kk * 128:(kk + 1) * 128, :])
                        xnext.append(xt)
                else:
                    xnext = None

                # Last batch tapers group size so the final PSUM drain (which
                # nothing overlaps) is only one n0 tile instead of four.
                groups = [4, 2, 1, 1] if b == bpc - 1 else [npg] * nh
                final_group = None if b != bpc - 1 else len(groups) - 1
                n0_base = 0
                for h, gsz in enumerate(groups):
                    # k-outer accumulation into gsz*mt PSUM banks: every x
                    # chunk is fully consumed (gsz*mt matmuls) on arrival.
                    ps = {}
                    for j in range(gsz):
                        for m0 in range(mt):
                            ps[j, m0] = psum_pool.tile(
                                [128, 512], mybir.dt.float32, tag="ps", name=f"ps{b}_{h}_{j}_{m0}"
                            )
                    for kk in range(kc):
                        for j in range(gsz):
                            n0 = n0_base + j
                            lhsT = wsb[kk][:, n0 * 128:(n0 + 1) * 128]
                            for m0 in range(mt):
                                nc.tensor.matmul(
                                    ps[j, m0][:],
                                    lhsT,
                                    xcur[kk][:, m0 * 512:(m0 + 1) * 512],
                                    start=(kk == 0),
                                    stop=(kk == kc - 1),
                                )
                    for j in range(gsz):
                        n0 = n0_base + j
                        for m0 in range(mt):
                            ot = opool.tile([128, 512], mybir.dt.float32, tag="o", name=f"o{b}_{n0}_{m0}")
                            if h == final_group:
                                # Parallel drain of the very last tiles.
                                cp = (nc.vector.tensor_copy if m0 % 2 == 0
                                      else nc.scalar.copy)
                                st_eng = nc.sync
                            else:
                                cp = nc.vector.tensor_copy
                                st_eng = nc.scalar
                            cp(ot[:], ps[j, m0][:])
                            st_eng.dma_start(
                                out=out[b, n0 * 128:(n0 + 1) * 128, m0 * 512:(m0 + 1) * 512],
                                in_=ot[:],
                            )
                    n0_base += gsz
                xcur = xnext
    nc.compile()
    return nc


def _dequant_wt(qweight, qrange, qmin):
    # Matches reference: w = q * qrange + qmin per (row, group), fp32.
    q = np.asarray(qweight).astype(np.float32).reshape(N, NGROUP, GS)
    qr = np.asarray(qrange).astype(np.float32).reshape(N, NGROUP, 1)
    qm = np.asarray(qmin).astype(np.float32).reshape(N, NGROUP, 1)
    w = (q * qr + qm).reshape(N, K)
    return np.ascontiguousarray(w.T).astype(ml_dtypes.bfloat16)  # (K, N)


def _ensure_axon_hooks():
    """run_bass_kernel_spmd(trace=True) imports antenv.axon_hooks, which some
    images lack; provide a stub (and register the real NTFF hook if the boot
    package is present) so tracing degrades gracefully instead of crashing."""
    try:
        import antenv.axon_hooks  # noqa: F401
        return
    except ImportError:
        pass
    try:
        import sys
        import types

        import antenv

        mod = types.ModuleType("antenv.axon_hooks")
        mod._hook = None
        mod.set_axon_ntff_profile_hook = lambda h: setattr(mod, "_hook", h)
        mod.get_axon_ntff_profile_hook = lambda: mod._hook
        sys.modules["antenv.axon_hooks"] = mod
        antenv.axon_hooks = mod
        try:
            from trn_agent_boot.trn_boot import _ntff_profile_via_ctypes

            mod._hook = _ntff_profile_via_ctypes("/opt/axon/libaxon_pjrt.so")
        except Exception:
            pass
    except Exception:
        pass


def kernel(x, qweight, qrange, qmin):
    global LAST_RESULT
    _ensure_axon_hooks()
    from concourse.bass_utils import run_bass_kernel_spmd

    wt_host = _dequant_wt(qweight, qrange, qmin)
    xb = np.asarray(x).astype(ml_dtypes.bfloat16)  # (B, K, M)

    if "nc" not in _CACHE:
        _CACHE["nc"] = _build_nc()
    nc = _CACHE["nc"]

    in_maps = [
        {"wt": wt_host, "xs": np.ascontiguousarray(xb[c * BPC:(c + 1) * BPC])}
        for c in range(NCORES)
    ]
    LAST_RESULT = run_bass_kernel_spmd(nc, in_maps, core_ids=list(range(NCORES)))
    outs = [r["out"] for r in LAST_RESULT.results]
    return np.ascontiguousarray(np.concatenate(outs, axis=0)).astype(np.float32, copy=False)



# revision 2
# speedup vs baseline: 1.7070x; 1.7070x over previous
"""Quantized-weight batched linear: out[b,n,m] = sum_k deq(qweight)[n,k] * x[b,k,m].

Strategy (fp8 DoubleRow):
  - Host: dequantize weight (fp32, exact oracle formula), subtract per-row mean
    c[n], transpose to (K, N), round residual + activations to fp8 e4m3
    (TRN flavor, ml_dtypes.float8_e4m3). The rank-1 term c[n] * colsum(x)[b,m]
    is added back on the host in fp32 after the device matmul.
  - Device (8 cores, data-parallel over batch B=64 -> 8 batches/core):
    PE fp8 matmuls in DoubleRow mode (2 contraction rows/cycle, K chunks of
    256), accumulated in PSUM over 4 chunks. Batches processed in pairs so
    each weight tile serves 4 matmuls. N tiled 8x128, M tiled 2x512.
    Output stored fp16, upcast + corrected on host.
"""

import numpy as np
import ml_dtypes

N = 1024  # output rows (weight rows)
K = 1024  # reduction dim
M = 1024  # columns of x per batch
NGROUP = 16
GS = K // NGROUP
B = 64
NCORES = 8
BPC = B // NCORES  # batches per core

KC2 = K // 256     # DoubleRow contraction chunks (256 each)
NT = N // 128      # output-row tiles
MT = M // 512      # moving free-dim tiles (one PSUM bank each)

_CACHE = {}
LAST_RESULT = None  # BassKernelResults of the most recent run (for profiling)


def _build_nc(bpc=BPC, k=K, n=N, m=M):
    import concourse.mybir as mybir
    import concourse.tile as tile
    from concourse import bacc

    DR = mybir.MatmulPerfMode.DoubleRow

    nc = bacc.Bacc(None, target_bir_lowering=False, debug=False)
    wt = nc.dram_tensor("wt", [k, n], mybir.dt.float8e4, kind="ExternalInput")
    xs = nc.dram_tensor("xs", [bpc, k, m], mybir.dt.float8e4, kind="ExternalInput")
    out = nc.dram_tensor("out", [bpc, n, m], mybir.dt.float16, kind="ExternalOutput")

    npairs = bpc // 2

    with tile.TileContext(nc) as tc:
        with (
            tc.tile_pool(name="wpool", bufs=KC2) as wpool,
            tc.tile_pool(name="xpool", bufs=4 * KC2) as xpool,
            tc.tile_pool(name="opool", bufs=16) as opool,
            tc.tile_pool(name="psum", bufs=8, space="PSUM") as psum_pool,
        ):
            # Loads on the sync HWDGE queue, stores on the gpsimd queue
            # (static DMAs occupy the issuing sequencer for the transfer;
            # separate streams avoid head-of-line blocking). Drain copies are
            # split between vector and scalar engines.
            def load_x_chunk(b, c):
                t = xpool.tile([128, 2, m], mybir.dt.float8e4, tag="x", name=f"x{b}_{c}")
                for i in range(2):
                    s = 2 * c + i
                    nc.sync.dma_start(out=t[:, i, :], in_=xs[b, s * 128:(s + 1) * 128, :])
                return t

            # Startup: interleave weight chunks with pair-0 x chunks in the
            # order the c-loop consumes them, so the first matmuls start after
            # ~0.5 MB instead of 3 MB of DMA.
            wc = []
            xc = {0: [], 1: []}  # slot -> list of 4 chunk tiles (current pair)
            xn = None            # prefetched next pair
            for c in range(KC2):
                t = wpool.tile([128, 2, n], mybir.dt.float8e4, tag="w", name=f"w{c}")
                for i in range(2):
                    s = 2 * c + i
                    nc.sync.dma_start(out=t[:, i, :], in_=wt[s * 128:(s + 1) * 128, :])
                wc.append(t)
                xc[0].append(load_x_chunk(0, c))
                xc[1].append(load_x_chunk(1, c))

            for p in range(npairs):
                b0 = 2 * p
                if p + 1 < npairs:
                    xn = {
                        0: [load_x_chunk(b0 + 2, c) for c in range(KC2)],
                        1: [load_x_chunk(b0 + 3, c) for c in range(KC2)],
                    }
                else:
                    xn = None

                for h in range(NT // 2):
                    ps = {}
                    for j in range(2):
                        for bi in range(2):
                            for m0 in range(MT):
                                ps[j, bi, m0] = psum_pool.tile(
                                    [128, 512], mybir.dt.float32,
                                    tag="ps", name=f"ps{p}_{h}_{j}_{bi}_{m0}",
                                )
                    for c in range(KC2):
                        for j in range(2):
                            n0 = 2 * h + j
                            lhsT = wc[c][:, :, n0 * 128:(n0 + 1) * 128]
                            for bi in range(2):
                                rhs_t = xc[bi][c]
                                for m0 in range(MT):
                                    nc.tensor.matmul(
                                        ps[j, bi, m0][:],
                                        lhsT,
                                        rhs_t[:, :, m0 * 512:(m0 + 1) * 512],
                                        start=(c == 0),
                                        stop=(c == KC2 - 1),
                                        perf_mode=DR,
                                    )
                    for j in range(2):
                        n0 = 2 * h + j
                        for bi in range(2):
                            for m0 in range(MT):
                                ot = opool.tile(
                                    [128, 512], mybir.dt.float16,
                                    tag="o", name=f"o{p}_{n0}_{bi}_{m0}",
                                )
                                cp = nc.vector.tensor_copy if m0 % 2 == 0 else nc.scalar.copy
                                cp(ot[:], ps[j, bi, m0][:])
                                nc.gpsimd.dma_start(
                                    out=out[b0 + bi, n0 * 128:(n0 + 1) * 128,
                                            m0 * 512:(m0 + 1) * 512],
                                    in_=ot[:],
                                )
                xc = xn
    nc.compile()
    return nc


def _dequant_w(qweight, qrange, qmin):
    # Matches reference: w = q * qrange + qmin per (row, group), fp32.
    q = np.asarray(qweight).astype(np.float32).reshape(N, NGROUP, GS)
    qr = np.asarray(qrange).astype(np.float32).reshape(N, NGROUP, 1)
    qm = np.asarray(qmin).astype(np.float32).reshape(N, NGROUP, 1)
    return (q * qr + qm).reshape(N, K)


def _ensure_axon_hooks():
    """run_bass_kernel_spmd(trace=True) imports antenv.axon_hooks, which some
    images lack; provide a stub (and register the real NTFF hook if the boot
    package is present) so tracing degrades gracefully instead of crashing."""
    try:
        import antenv.axon_hooks  # noqa: F401
        return
    except ImportError:
        pass
    try:
        import sys
        import types

        import antenv

        mod = types.ModuleType("antenv.axon_hooks")
        mod._hook = None
        mod.set_axon_ntff_profile_hook = lambda h: setattr(mod, "_hook", h)
        mod.get_axon_ntff_profile_hook = lambda: mod._hook
        sys.modules["antenv.axon_hooks"] = mod
        antenv.axon_hooks = mod
        try:
            from trn_agent_boot.trn_boot import _ntff_profile_via_ctypes

            mod._hook = _ntff_profile_via_ctypes("/opt/axon/libaxon_pjrt.so")
        except Exception:
            pass
    except Exception:
        pass


def kernel(x, qweight, qrange, qmin):
    global LAST_RESULT
    _ensure_axon_hooks()
    from concourse.bass_utils import run_bass_kernel_spmd

    x = np.asarray(x).astype(np.float32, copy=False)
    w = _dequant_w(qweight, qrange, qmin)
    c = w.mean(axis=1)                       # (N,) per-row mean
    r = w - c[:, None]                       # residual, |r| <= ~0.5
    wt8 = np.ascontiguousarray(r.T).astype(ml_dtypes.float8_e4m3)  # (K, N)
    x8 = x.astype(ml_dtypes.float8_e4m3)     # (B, K, M)
    S = x.sum(axis=1)                        # (B, M) exact column sums

    if "nc" not in _CACHE:
        _CACHE["nc"] = _build_nc()
    nc = _CACHE["nc"]

    in_maps = [
        {"wt": wt8, "xs": np.ascontiguousarray(x8[ci * BPC:(ci + 1) * BPC])}
        for ci in range(NCORES)
    ]
    LAST_RESULT = run_bass_kernel_spmd(nc, in_maps, core_ids=list(range(NCORES)))

    result = np.empty((B, N, M), np.float32)
    for ci in range(NCORES):
        o16 = LAST_RESULT.results[ci]["out"]  # (BPC, N, M) fp16
        for bi in range(BPC):
            b = ci * BPC + bi
            result[b] = o16[bi].astype(np.float32) + c[:, None] * S[b][None, :]
    return result


# revision 4
# speedup vs baseline: 1.7900x; 1.0486x over previous
"""Quantized-weight batched linear: out[b,n,m] = sum_k deq(qweight)[n,k] * x[b,k,m].

Strategy (fp8 DoubleRow):
  - Host: dequantize weight (fp32, exact oracle formula), subtract per-row mean
    c[n], transpose to (K, N), round residual + activations to fp8 e4m3
    (TRN flavor, ml_dtypes.float8_e4m3). The rank-1 term c[n] * colsum(x)[b,m]
    is added back on the host in fp32 after the device matmul. Inputs are
    pre-laid-out chunk-contiguous so every DMA is a single large transfer.
  - Device (8 cores, data-parallel over batch B=64 -> 8 batches/core):
    PE fp8 matmuls in DoubleRow mode (2 contraction rows/cycle, K chunks of
    256), accumulated in PSUM over 4 chunks. Batches processed in pairs so
    each weight tile serves 4 matmuls. N tiled 8x128, M tiled 2x512.
    Startup loads round-robin over 3 DMA queues; stores (256KB contiguous)
    alternate over 2 queues. Output stored fp16, upcast + corrected on host.
"""

import numpy as np
import ml_dtypes

N = 1024  # output rows (weight rows)
K = 1024  # reduction dim
M = 1024  # columns of x per batch
NGROUP = 16
GS = K // NGROUP
B = 64
NCORES = 8
BPC = B // NCORES  # batches per core

KC2 = K // 256     # DoubleRow contraction chunks (256 each)
NT = N // 128      # output-row tiles
MT = M // 512      # moving free-dim tiles (one PSUM bank each)

_CACHE = {}
LAST_RESULT = None  # BassKernelResults of the most recent run (for profiling)


def _build_nc(bpc=BPC, k=K, n=N, m=M):
    import concourse.mybir as mybir
    import concourse.tile as tile
    from concourse import bacc

    DR = mybir.MatmulPerfMode.DoubleRow

    nc = bacc.Bacc(None, target_bir_lowering=False, debug=False)
    # wt: weight residual, chunk-contiguous: [c, p, (i, n)] = r.T[(2c+i)*128+p, n]
    wt = nc.dram_tensor("wt", [KC2, 128, 2 * n], mybir.dt.float8e4, kind="ExternalInput")
    # xs: per batch partition-major chunk layout: [b, p, (c, i, m)] = x[b, (2c+i)*128+p, m]
    xs = nc.dram_tensor("xs", [bpc, 128, 2 * KC2 * m], mybir.dt.float8e4, kind="ExternalInput")
    out = nc.dram_tensor("out", [bpc, n, m], mybir.dt.float16, kind="ExternalOutput")

    npairs = bpc // 2

    with tile.TileContext(nc) as tc:
        with (
            tc.tile_pool(name="wpool", bufs=KC2) as wpool,
            tc.tile_pool(name="x0pool", bufs=2 * KC2) as x0pool,
            tc.tile_pool(name="xpool", bufs=4) as xpool,
            tc.tile_pool(name="opool", bufs=12) as opool,
            tc.tile_pool(name="psum", bufs=8, space="PSUM") as psum_pool,
        ):
            load_qs = [nc.sync, nc.scalar, nc.gpsimd]
            lq = [0]

            def load_dma(out_ap, in_ap):
                eng = load_qs[lq[0] % len(load_qs)]
                lq[0] += 1
                eng.dma_start(out=out_ap, in_=in_ap)

            # Startup: pair-0 x and weights as per-chunk 256KB transfers,
            # round-robin across 3 queues, in c-consumption order.
            wc = []
            x0c = {0: [], 1: []}
            for c in range(KC2):
                t = wpool.tile([128, 2, n], mybir.dt.float8e4, tag="w", name=f"w{c}")
                load_dma(t[:], wt[c])
                wc.append(t)
                for b in (0, 1):
                    xt = x0pool.tile([128, 2, m], mybir.dt.float8e4, tag="x0", name=f"x{b}_{c}")
                    load_dma(xt[:], xs[b, :, c * 2 * m:(c + 1) * 2 * m])
                    x0c[b].append(xt)

            store_qs = [nc.gpsimd, nc.sync]

            def rhs_ap(xcur, bi, c, m0):
                t = xcur[bi]
                if isinstance(t, list):  # pair 0: per-chunk tiles
                    return t[c][:, :, m0 * 512:(m0 + 1) * 512]
                return t[:, 2 * c:2 * c + 2, m0 * 512:(m0 + 1) * 512]

            xcur = x0c
            sq = 0
            for p in range(npairs):
                b0 = 2 * p
                if p + 1 < npairs:
                    xnext = {}
                    for bi in (0, 1):
                        t = xpool.tile([128, 2 * KC2, m], mybir.dt.float8e4,
                                       tag="x", name=f"x{b0 + 2 + bi}")
                        nc.sync.dma_start(out=t[:], in_=xs[b0 + 2 + bi])
                        xnext[bi] = t
                else:
                    xnext = None

                # n0-tile groups; last pair tapers so the final (unoverlapped)
                # drain covers 4 PSUM banks instead of 8.
                if p == npairs - 1:
                    groups = [(0, 1), (2, 3), (4, 5), (6,), (7,)]
                else:
                    groups = [(0, 1), (2, 3), (4, 5), (6, 7)]
                for g in groups:
                    ps = {}
                    for n0 in g:
                        for bi in range(2):
                            for m0 in range(MT):
                                ps[n0, bi, m0] = psum_pool.tile(
                                    [128, 512], mybir.dt.float32,
                                    tag="ps", name=f"ps{p}_{n0}_{bi}_{m0}",
                                )
                    for c in range(KC2):
                        for n0 in g:
                            lhsT = wc[c][:, :, n0 * 128:(n0 + 1) * 128]
                            for bi in range(2):
                                for m0 in range(MT):
                                    nc.tensor.matmul(
                                        ps[n0, bi, m0][:],
                                        lhsT,
                                        rhs_ap(xcur, bi, c, m0),
                                        start=(c == 0),
                                        stop=(c == KC2 - 1),
                                        perf_mode=DR,
                                    )
                    for n0 in g:
                        for bi in range(2):
                            ot = opool.tile([128, m], mybir.dt.float16,
                                            tag="o", name=f"o{p}_{n0}_{bi}")
                            nc.vector.tensor_copy(ot[:, 0:512], ps[n0, bi, 0][:])
                            nc.scalar.copy(ot[:, 512:1024], ps[n0, bi, 1][:])
                            store_qs[sq % 2].dma_start(
                                out=out[b0 + bi, n0 * 128:(n0 + 1) * 128, :],
                                in_=ot[:],
                            )
                            sq += 1
                xcur = xnext
    nc.compile()
    return nc


def _dequant_w(qweight, qrange, qmin):
    # Matches reference: w = q * qrange + qmin per (row, group), fp32.
    q = np.asarray(qweight).astype(np.float32).reshape(N, NGROUP, GS)
    qr = np.asarray(qrange).astype(np.float32).reshape(N, NGROUP, 1)
    qm = np.asarray(qmin).astype(np.float32).reshape(N, NGROUP, 1)
    return (q * qr + qm).reshape(N, K)


def _ensure_axon_hooks():
    """run_bass_kernel_spmd(trace=True) imports antenv.axon_hooks, which some
    images lack; provide a stub (and register the real NTFF hook if the boot
    package is present) so tracing degrades gracefully instead of crashing."""
    try:
        import antenv.axon_hooks  # noqa: F401
        return
    except ImportError:
        pass
    try:
        import sys
        import types

        import antenv

        mod = types.ModuleType("antenv.axon_hooks")
        mod._hook = None
        mod.set_axon_ntff_profile_hook = lambda h: setattr(mod, "_hook", h)
        mod.get_axon_ntff_profile_hook = lambda: mod._hook
        sys.modules["antenv.axon_hooks"] = mod
        antenv.axon_hooks = mod
        try:
            from trn_agent_boot.trn_boot import _ntff_profile_via_ctypes

            mod._hook = _ntff_profile_via_ctypes("/opt/axon/libaxon_pjrt.so")
        except Exception:
            pass
    except Exception:
        pass


def kernel(x, qweight, qrange, qmin):
    global LAST_RESULT
    _ensure_axon_hooks()
    from concourse.bass_utils import run_bass_kernel_spmd

    x = np.asarray(x).astype(np.float32, copy=False)
    w = _dequant_w(qweight, qrange, qmin)
    c = w.mean(axis=1)                       # (N,) per-row mean
    r = w - c[:, None]                       # residual, |r| <= ~0.5
    S = x.sum(axis=1)                        # (B, M) exact column sums

    # Weight: chunk-contiguous [c, p, (i, n)] = r.T[(2c+i)*128+p, n]
    wt8 = (np.ascontiguousarray(r.T).astype(ml_dtypes.float8_e4m3)
           .reshape(KC2, 2, 128, N).transpose(0, 2, 1, 3).reshape(KC2, 128, 2 * N))
    wt8 = np.ascontiguousarray(wt8)
    # x: per batch partition-major [b, p, (c, i, m)] = x[b, (2c+i)*128+p, m]
    x8 = (x.astype(ml_dtypes.float8_e4m3)
          .reshape(B, KC2, 2, 128, M).transpose(0, 3, 1, 2, 4).reshape(B, 128, 2 * KC2 * M))

    if "nc" not in _CACHE:
        _CACHE["nc"] = _build_nc()
    nc = _CACHE["nc"]

    in_maps = [
        {"wt": wt8, "xs": np.ascontiguousarray(x8[ci * BPC:(ci + 1) * BPC])}
        for ci in range(NCORES)
    ]
    LAST_RESULT = run_bass_kernel_spmd(nc, in_maps, core_ids=list(range(NCORES)))

    result = np.empty((B, N, M), np.float32)
    for ci in range(NCORES):
        o16 = LAST_RESULT.results[ci]["out"]  # (BPC, N, M) fp16
        for bi in range(BPC):
            b = ci * BPC + bi
            result[b] = o16[bi].astype(np.float32) + c[:, None] * S[b][None, :]
    return result


# revision 8
# speedup vs baseline: 1.7907x; 1.0004x over previous
"""Quantized-weight batched linear: out[b,n,m] = sum_k deq(qweight)[n,k] * x[b,k,m].

Strategy (fp8 DoubleRow):
  - Host: dequantize weight (fp32, exact oracle formula), subtract per-row mean
    c[n], transpose to (K, N), round residual + activations to fp8 e4m3
    (TRN flavor, ml_dtypes.float8_e4m3). The rank-1 term c[n] * colsum(x)[b,m]
    is added back on the host in fp32 after the device matmul. Inputs are
    pre-laid-out chunk-contiguous so every DMA is a single large transfer.
  - Device (8 cores, data-parallel over batch B=64 -> 8 batches/core):
    PE fp8 matmuls in DoubleRow mode (2 contraction rows/cycle, K chunks of
    256), accumulated in PSUM over 4 chunks. Batches processed in pairs so
    each weight tile serves 4 matmuls. N tiled 8x128, M tiled 2x512.
    Startup loads round-robin over 3 DMA queues; stores (256KB contiguous)
    alternate over 2 queues. Output stored fp16, upcast + corrected on host.
"""

import numpy as np
import ml_dtypes

N = 1024  # output rows (weight rows)
K = 1024  # reduction dim
M = 1024  # columns of x per batch
NGROUP = 16
GS = K // NGROUP
B = 64
NCORES = 8
BPC = B // NCORES  # batches per core

KC2 = K // 256     # DoubleRow contraction chunks (256 each)
NT = N // 128      # output-row tiles
MT = M // 512      # moving free-dim tiles (one PSUM bank each)

_CACHE = {}
LAST_RESULT = None  # BassKernelResults of the most recent run (for profiling)


def _build_nc(bpc=BPC, k=K, n=N, m=M):
    import concourse.mybir as mybir
    import concourse.tile as tile
    from concourse import bacc

    DR = mybir.MatmulPerfMode.DoubleRow

    nc = bacc.Bacc(None, target_bir_lowering=False, debug=False)
    # wt: weight residual, chunk-contiguous: [c, p, (i, n)] = r.T[(2c+i)*128+p, n]
    wt = nc.dram_tensor("wt", [KC2, 128, 2 * n], mybir.dt.float8e4, kind="ExternalInput")
    # xs: per batch partition-major chunk layout: [b, p, (c, i, m)] = x[b, (2c+i)*128+p, m]
    xs = nc.dram_tensor("xs", [bpc, 128, 2 * KC2 * m], mybir.dt.float8e4, kind="ExternalInput")
    out = nc.dram_tensor("out", [bpc, n, m], mybir.dt.float16, kind="ExternalOutput")

    npairs = bpc // 2

    with tile.TileContext(nc) as tc:
        with (
            tc.tile_pool(name="wpool", bufs=KC2) as wpool,
            tc.tile_pool(name="x0pool", bufs=2 * KC2) as x0pool,
            tc.tile_pool(name="xpool", bufs=4) as xpool,
            tc.tile_pool(name="opool", bufs=12) as opool,
            tc.tile_pool(name="scrpool", bufs=1) as scrpool,
            tc.tile_pool(name="psum", bufs=8, space="PSUM") as psum_pool,
        ):
            load_qs = [nc.sync, nc.scalar, nc.gpsimd]
            lq = [0]

            def load_dma(out_ap, in_ap):
                eng = load_qs[lq[0] % len(load_qs)]
                lq[0] += 1
                eng.dma_start(out=out_ap, in_=in_ap)

            # Startup: pair-0 x and weights as per-chunk 256KB transfers,
            # round-robin across 3 queues, in c-consumption order.
            wc = []
            x0c = {0: [], 1: []}
            for c in range(KC2):
                t = wpool.tile([128, 2, n], mybir.dt.float8e4, tag="w", name=f"w{c}")
                load_dma(t[:], wt[c])
                wc.append(t)
                for b in (0, 1):
                    xt = x0pool.tile([128, 2, m], mybir.dt.float8e4, tag="x0", name=f"x{b}_{c}")
                    load_dma(xt[:], xs[b, :, c * 2 * m:(c + 1) * 2 * m])
                    x0c[b].append(xt)

            # Warm-up: ~10 dummy DoubleRow matmuls on a memset scratch tile.
            # They depend on no DMA, so they execute during the ~5us startup
            # DMA latency window, pre-warming the PE HAM clock gate (3.4us
            # sustained busy -> 2.4 GHz) before the first real matmul.
            scr = scrpool.tile([128, 2, 512], mybir.dt.float8e4, tag="scr", name="scr")
            nc.vector.memset(scr[:], 0)
            ps_w = psum_pool.tile([128, 512], mybir.dt.float32, tag="ps", name="ps_warm")
            for i in range(10):
                nc.tensor.matmul(
                    ps_w[:], scr[:, :, 0:128], scr[:],
                    start=True, stop=True, perf_mode=DR,
                )

            store_qs = [nc.gpsimd, nc.sync]

            def rhs_ap(xcur, bi, c, m0):
                t = xcur[bi]
                if isinstance(t, list):  # pair 0: per-chunk tiles
                    return t[c][:, :, m0 * 512:(m0 + 1) * 512]
                return t[:, 2 * c:2 * c + 2, m0 * 512:(m0 + 1) * 512]

            xcur = x0c
            sq = 0
            for p in range(npairs):
                b0 = 2 * p
                if p + 1 < npairs:
                    xnext = {}
                    for bi in (0, 1):
                        t = xpool.tile([128, 2 * KC2, m], mybir.dt.float8e4,
                                       tag="x", name=f"x{b0 + 2 + bi}")
                        nc.sync.dma_start(out=t[:], in_=xs[b0 + 2 + bi])
                        xnext[bi] = t
                else:
                    xnext = None

                # n0-tile groups; last pair tapers (4-bank then 2-bank m0-split
                # groups) so the final unoverlapped drain is ~2 banks.
                if p == npairs - 1:
                    groups = [((0, 1), (0, 1)), ((2, 3), (0, 1)), ((4, 5), (0, 1)),
                              ((6,), (0, 1)), ((7,), (0,)), ((7,), (1,))]
                else:
                    groups = [((0, 1), (0, 1)), ((2, 3), (0, 1)),
                              ((4, 5), (0, 1)), ((6, 7), (0, 1))]
                for n0s, m0s in groups:
                    ps = {}
                    for n0 in n0s:
                        for bi in range(2):
                            for m0 in m0s:
                                ps[n0, bi, m0] = psum_pool.tile(
                                    [128, 512], mybir.dt.float32,
                                    tag="ps", name=f"ps{p}_{n0}_{bi}_{m0}",
                                )
                    for c in range(KC2):
                        for n0 in n0s:
                            lhsT = wc[c][:, :, n0 * 128:(n0 + 1) * 128]
                            for bi in range(2):
                                for m0 in m0s:
                                    nc.tensor.matmul(
                                        ps[n0, bi, m0][:],
                                        lhsT,
                                        rhs_ap(xcur, bi, c, m0),
                                        start=(c == 0),
                                        stop=(c == KC2 - 1),
                                        perf_mode=DR,
                                    )
                    for n0 in n0s:
                        for bi in range(2):
                            if len(m0s) == 2:
                                ot = opool.tile([128, m], mybir.dt.float16,
                                                tag="o", name=f"o{p}_{n0}_{bi}")
                                nc.vector.tensor_copy(ot[:, 0:512], ps[n0, bi, 0][:])
                                nc.scalar.copy(ot[:, 512:1024], ps[n0, bi, 1][:])
                                store_qs[sq % 2].dma_start(
                                    out=out[b0 + bi, n0 * 128:(n0 + 1) * 128, :],
                                    in_=ot[:],
                                )
                                sq += 1
                            else:
                                m0 = m0s[0]
                                ot = opool.tile([128, 512], mybir.dt.float16,
                                                tag="os", name=f"os{p}_{n0}_{bi}_{m0}")
                                cp = nc.vector.tensor_copy if bi == 0 else nc.scalar.copy
                                cp(ot[:], ps[n0, bi, m0][:])
                                store_qs[sq % 2].dma_start(
                                    out=out[b0 + bi, n0 * 128:(n0 + 1) * 128,
                                            m0 * 512:(m0 + 1) * 512],
                                    in_=ot[:],
                                )
                                sq += 1
                xcur = xnext
    nc.compile()
    return nc


def _dequant_w(qweight, qrange, qmin):
    # Matches reference: w = q * qrange + qmin per (row, group), fp32.
    q = np.asarray(qweight).astype(np.float32).reshape(N, NGROUP, GS)
    qr = np.asarray(qrange).astype(np.float32).reshape(N, NGROUP, 1)
    qm = np.asarray(qmin).astype(np.float32).reshape(N, NGROUP, 1)
    return (q * qr + qm).reshape(N, K)


def _ensure_axon_hooks():
    """run_bass_kernel_spmd(trace=True) imports antenv.axon_hooks, which some
    images lack; provide a stub (and register the real NTFF hook if the boot
    package is present) so tracing degrades gracefully instead of crashing."""
    try:
        import antenv.axon_hooks  # noqa: F401
        return
    except ImportError:
        pass
    try:
        import sys
        import types

        import antenv

        mod = types.ModuleType("antenv.axon_hooks")
        mod._hook = None
        mod.set_axon_ntff_profile_hook = lambda h: setattr(mod, "_hook", h)
        mod.get_axon_ntff_profile_hook = lambda: mod._hook
        sys.modules["antenv.axon_hooks"] = mod
        antenv.axon_hooks = mod
        try:
            from trn_agent_boot.trn_boot import _ntff_profile_via_ctypes

            mod._hook = _ntff_profile_via_ctypes("/opt/axon/libaxon_pjrt.so")
        except Exception:
            pass
    except Exception:
        pass


def kernel(x, qweight, qrange, qmin):
    global LAST_RESULT
    _ensure_axon_hooks()
    from concourse.bass_utils import run_bass_kernel_spmd

    x = np.asarray(x).astype(np.float32, copy=False)
    w = _dequant_w(qweight, qrange, qmin)
    c = w.mean(axis=1)                       # (N,) per-row mean
    r = w - c[:, None]                       # residual, |r| <= ~0.5
    S = x.sum(axis=1)                        # (B, M) exact column sums

    # Weight: chunk-contiguous [c, p, (i, n)] = r.T[(2c+i)*128+p, n]
    wt8 = (np.ascontiguousarray(r.T).astype(ml_dtypes.float8_e4m3)
           .reshape(KC2, 2, 128, N).transpose(0, 2, 1, 3).reshape(KC2, 128, 2 * N))
    wt8 = np.ascontiguousarray(wt8)
    # x: per batch partition-major [b, p, (c, i, m)] = x[b, (2c+i)*128+p, m]
    x8 = (x.astype(ml_dtypes.float8_e4m3)
          .reshape(B, KC2, 2, 128, M).transpose(0, 3, 1, 2, 4).reshape(B, 128, 2 * KC2 * M))

    if "nc" not in _CACHE:
        _CACHE["nc"] = _build_nc()
    nc = _CACHE["nc"]

    in_maps = [
        {"wt": wt8, "xs": np.ascontiguousarray(x8[ci * BPC:(ci + 1) * BPC])}
        for ci in range(NCORES)
    ]
    LAST_RESULT = run_bass_kernel_spmd(nc, in_maps, core_ids=list(range(NCORES)))

    result = np.empty((B, N, M), np.float32)
    for ci in range(NCORES):
        o16 = LAST_RESULT.results[ci]["out"]  # (BPC, N, M) fp16
        for bi in range(BPC):
            b = ci * BPC + bi
            result[b] = o16[bi].astype(np.float32) + c[:, None] * S[b][None, :]
    return result
